# revision 54
# baseline (speedup 1.0000x reference)
"""Trainium2 Bass kernel for DiagonalSelectiveCell.

Problem:
    delta = sigmoid(x @ Wd^T + b_delta)        [T,B,D]
    cand  = x @ Wx^T + b                       [T,B,D]
    gate  = silu(x @ Wg^T + b_gate)            [T,B,D]
    scan over t:  h_t = (1-delta_t) * h_{t-1} + delta_t * tanh(cand_t + r_h*h_{t-1})
    output = h_seq * gate;  h = concat([h0], h_seq)
Returns (output [T,B,D], h [T+1,B,D]).

Strategy (8 NeuronCores, batch-parallel):
  - Shard B=16 across 8 cores (2 rows each), replicate weights. No collectives.
  - Host pre-transposes x to [B_local, D, T] so the GEMMs need no on-device
    transpose: channels live on SBUF partitions, time on the free axis.
  - For the staged inputs r_h == 0, so the recurrence is a first-order LINEAR
    scan per (b, d) lane:  h_t = a_t*h_{t-1} + u_t  with a = 1-delta,
    u = delta*tanh(cand).  That maps 1:1 onto the hardware
    `tensor_tensor_scan` instruction (one instruction scans 2048 steps for
    128 lanes).  A nonzero r_h falls back to an exact numpy path.
  - Per core: for each batch row and each 128-channel tile, fp32r matmuls
    accumulate z in PSUM (K=1024 in 8 chunks, N=512 token tiles), ACT applies
    sigmoid/tanh (one table set, no table swaps; silu is computed as
    z*sigmoid(z) with a DVE scalar_tensor_tensor), DVE runs the scan and the
    output gating, and results DMA out in [D, T] layout (host transposes back).
"""

import numpy as np

T, B, D = 2048, 16, 1024
NCORES = 8
BL = B // NCORES  # batch rows per core
P = 128           # partition tile (channels)
NT = 512          # token tile (PSUM bank / moving free dim)

_NC_CACHE = {}


def _build_nc(t, d, bl, nt):
    import concourse.mybir as mybir
    from concourse import bacc
    from concourse.tile import TileContext

    F32 = mybir.dt.float32
    F32R = mybir.dt.float32r
    AF = mybir.ActivationFunctionType
    OP = mybir.AluOpType

    kd_n = d // P   # contraction chunks
    ke_n = d // P   # output-channel tiles
    ntok = t // nt  # token tiles

    nc = bacc.Bacc()
    xt = nc.declare_dram_parameter("xt", [bl, d, t], F32R, isOutput=False)
    wT = nc.declare_dram_parameter("wT", [3, d, d], F32R, isOutput=False)
    bias = nc.declare_dram_parameter("bias", [4, d], F32, isOutput=False)
    h0t = nc.declare_dram_parameter("h0t", [bl, d], F32, isOutput=False)
    outT = nc.declare_dram_parameter("outT", [bl, d, t], F32, isOutput=True)
    hT = nc.declare_dram_parameter("hT", [bl, d, t + 1], F32, isOutput=True)

    with TileContext(nc) as tc:
        with (
            tc.tile_pool(name="xpool", bufs=1) as xpool,
            tc.tile_pool(name="wpool", bufs=2) as wpool,
            tc.tile_pool(name="spool", bufs=2) as spool,
            tc.tile_pool(name="epool", bufs=2) as epool,
            tc.tile_pool(name="pspool", bufs=2, space="PSUM") as pspool,
        ):
            # bias/h0 tables loaded once: [P, ke] with element (p, ke) =
            # vec[ke*P + p]; per-e-tile slices are [P, 1] scalar APs
            btab = []
            for j in range(4):
                bt = spool.tile([P, ke_n], F32, tag=f"btab{j}", name=f"btab{j}", bufs=1)
                nc.sync.dma_start(out=bt[:], in_=bias[j].rearrange("(ke p) -> p ke", p=P))
                btab.append(bt)
            h0tab = []
            for bb in range(bl):
                ht0 = spool.tile([P, ke_n], F32, tag=f"h0tab{bb}", name=f"h0tab{bb}", bufs=1)
                nc.sync.dma_start(out=ht0[:], in_=h0t[bb].rearrange("(ke p) -> p ke", p=P))
                h0tab.append(ht0)
            for b in range(bl):
                x_sb = [
                    xpool.tile([P, t], F32R, tag=f"x{kd}", name=f"x{kd}", bufs=(2 if kd < 5 else 1))
                    for kd in range(kd_n)
                ]
                for ke in range(ke_n):
                    es = slice(ke * P, (ke + 1) * P)
                    w_sb = []
                    wvs = []
                    for w in range(3):
                        wtile = wpool.tile([P, kd_n * P], F32R, tag=f"w{w}")
                        wvs.append(wtile[:].rearrange("p (kd e) -> p kd e", e=P))
                        w_sb.append(wtile)
                    # Issue order matters: the sync HWDGE ring drains FIFO, so
                    # interleave per-kd weight chunks (needed first by the
                    # matmuls) with the big x chunks instead of queuing 8.4MB
                    # of x ahead of them.
                    if ke > 1:
                        # steady state: one ring slot per weight matrix (the
                        # per-dma fixed cost dominates over transfer size)
                        for w in range(3):
                            nc.sync.dma_start(
                                out=wvs[w][:],
                                in_=wT[w, :, es].rearrange("(kd p) e -> p kd e", p=P),
                            )
                    for kd in range(kd_n):
                        if ke <= 1:
                            for w in range(3):
                                # during the x burst the sync ring is taken;
                                # stream fine-grained weight chunks via SWDGE
                                nc.gpsimd.dma_start(out=wvs[w][:, kd, :], in_=wT[w, kd * P:(kd + 1) * P, es])
                    if ke == 0:
                        # x pieces kd-major: arrival order matches the
                        # kd-outer matmul sweep; per-token-tile pieces so
                        # each matmul unblocks as soon as its piece lands
                        for kd in range(kd_n):
                            for ntk in range(ntok):
                                nc.sync.dma_start(
                                    out=x_sb[kd][:, ntk * nt:(ntk + 1) * nt],
                                    in_=xt[b, kd * P:(kd + 1) * P, ntk * nt:(ntk + 1) * nt],
                                )
                    bd = btab[0][:, ke:ke + 1]
                    nbd = btab[1][:, ke:ke + 1]
                    bc = btab[2][:, ke:ke + 1]
                    bg = btab[3][:, ke:ke + 1]
                    h0_sb = h0tab[b][:, ke:ke + 1]

                    d_sb = epool.tile([P, t], F32, tag="d")
                    a_sb = epool.tile([P, t], F32, tag="a")
                    u_sb = epool.tile([P, t], F32, tag="u")
                    h_sb = epool.tile([P, t + 1], F32, tag="h")

                    def epilogue(w, ntk, ps, fuse_out=True):
                        ts_ = slice(ntk * nt, (ntk + 1) * nt)
                        if w == 0:
                            nc.scalar.activation(d_sb[:, ts_], ps[:], AF.Sigmoid, bias=bd, scale=1.0)
                            nc.scalar.activation(a_sb[:, ts_], ps[:], AF.Sigmoid, bias=nbd, scale=-1.0)
                        elif w == 1:
                            nc.scalar.activation(u_sb[:, ts_], ps[:], AF.Tanh, bias=bc, scale=1.0)
                            nc.vector.tensor_mul(u_sb[:, ts_], u_sb[:, ts_], d_sb[:, ts_])
                        else:
                            g_nt = epool.tile([P, nt], F32, tag="g", name="g_nt")
                            o_nt = epool.tile([P, nt], F32, tag="o", name="o_nt")
                            nc.scalar.activation(g_nt[:], ps[:], AF.Sigmoid, bias=bg, scale=1.0)
                            # gate = (z + b_gate) * sigmoid(z + b_gate)  (= silu)
                            nc.vector.scalar_tensor_tensor(
                                g_nt[:], ps[:], bg, g_nt[:],
                                op0=OP.add, op1=OP.mult,
                            )
                            # output slice pipelines right behind the gate
                            hs_ = slice(ntk * nt + 1, (ntk + 1) * nt + 1)
                            nc.vector.tensor_mul(o_nt[:], h_sb[:, hs_], g_nt[:])
                            nc.gpsimd.dma_start(out=outT[b, es, ts_], in_=o_nt[:])

                    def mm(ps, w, kd, ntk):
                        nc.tensor.matmul(
                            ps[:],
                            lhsT=w_sb[w][:, kd * P:(kd + 1) * P],
                            rhs=x_sb[kd][:, ntk * nt:(ntk + 1) * nt],
                            start=(kd == 0),
                            stop=(kd == kd_n - 1),
                        )

                    # w order: 0=delta(Wd), 1=cand(Wx), 2=gate(Wg)
                    w_list = (0, 1, 2)
                    for w in w_list:
                        # kd-outer: the same 128x128 weight tile feeds all 4
                        # token tiles (amortizes LDWEIGHTS), and the first
                        # matmul only needs the first x/w chunks in SBUF.
                        pss = [
                            pspool.tile([P, nt], F32, tag=f"ps{i}", name=f"ps{i}")
                            for i in range(ntok)
                        ]
                        for kd in range(kd_n):
                            for ntk in range(ntok):
                                mm(pss[ntk], w, kd, ntk)
                        for ntk in range(ntok):
                            epilogue(w, ntk, pss[ntk])
                        if w == 1:
                            # scan as soon as a,u are complete (overlaps the
                            # gate matmul pass on PE); h_sb[:, 0] carries h0
                            nc.vector.tensor_copy(h_sb[:, 0:1], h0_sb)
                            nc.vector.tensor_tensor_scan(
                                h_sb[:, 1:], a_sb[:], u_sb[:], h0_sb,
                                op0=OP.mult, op1=OP.add,
                            )
                    nc.gpsimd.dma_start(out=hT[b, es, :], in_=h_sb[:])
    nc.compile()
    return nc


def _get_nc(t=T, d=D, bl=BL, nt=NT):
    key = (t, d, bl, nt)
    if key not in _NC_CACHE:
        _NC_CACHE[key] = _build_nc(t, d, bl, nt)
    return _NC_CACHE[key]


def _marshal_inputs(x, h0, W_x, W_delta, W_gate, b, b_delta, b_gate):
    """Build the per-core input dicts (host-side shard + transpose)."""
    wT = np.ascontiguousarray(
        np.stack([np.asarray(W_delta).T, np.asarray(W_x).T, np.asarray(W_gate).T])
    ).astype(np.float32, copy=False)
    bias = np.ascontiguousarray(
        np.stack([b_delta, -np.asarray(b_delta), b, b_gate])
    ).astype(np.float32, copy=False)
    in_maps = []
    for c in range(NCORES):
        xs = np.ascontiguousarray(
            np.asarray(x)[:, c * BL:(c + 1) * BL, :].transpose(1, 2, 0)
        ).astype(np.float32, copy=False)
        h0s = np.ascontiguousarray(np.asarray(h0)[c * BL:(c + 1) * BL, :]).astype(
            np.float32, copy=False
        )
        in_maps.append({"xt": xs, "wT": wT, "bias": bias, "h0t": h0s})
    return in_maps


def _gather_outputs(results):
    output = np.empty((T, B, D), np.float32)
    h = np.empty((T + 1, B, D), np.float32)
    for c in range(NCORES):
        output[:, c * BL:(c + 1) * BL, :] = results[c]["outT"].transpose(2, 0, 1)
        h[:, c * BL:(c + 1) * BL, :] = results[c]["hT"].transpose(2, 0, 1)
    return output, h


def _run_device(in_maps, trace=False):
    from concourse.bass_utils import run_bass_kernel_spmd

    nc = _get_nc()
    return run_bass_kernel_spmd(
        nc, in_maps, core_ids=list(range(NCORES)), trace=trace
    )


def _numpy_fallback(x, h0, W_x, r_h, W_delta, W_gate, b, b_delta, b_gate):
    x = np.asarray(x, np.float32)
    delta = 1.0 / (1.0 + np.exp(-(np.einsum("tbd,ed->tbe", x, W_delta) + b_delta)))
    cand_x = np.einsum("tbd,ed->tbe", x, W_x) + b
    zg = np.einsum("tbd,ed->tbe", x, W_gate) + b_gate
    gate = zg / (1.0 + np.exp(-zg))
    h_seq = np.empty_like(delta)
    h_prev = np.asarray(h0, np.float32)
    for ti in range(x.shape[0]):
        cand = np.tanh(cand_x[ti] + np.asarray(r_h) * h_prev)
        h_prev = (1.0 - delta[ti]) * h_prev + delta[ti] * cand
        h_seq[ti] = h_prev
    output = h_seq * gate
    h = np.concatenate([np.asarray(h0, np.float32)[None], h_seq], axis=0)
    return output.astype(np.float32), h.astype(np.float32)


def kernel(x, h0, W_x, r_h, W_delta, W_gate, b, b_delta, b_gate):
    x = np.asarray(x, np.float32)
    assert x.shape == (T, B, D), f"unexpected x shape {x.shape}"
    if np.any(np.asarray(r_h) != 0):
        # recurrence is only linear (device-scannable) when r_h == 0
        return _numpy_fallback(x, h0, W_x, r_h, W_delta, W_gate, b, b_delta, b_gate)
    in_maps = _marshal_inputs(x, h0, W_x, W_delta, W_gate, b, b_delta, b_gate)
    res = _run_device(in_maps, trace=False)
    return _gather_outputs(res.results)


# revision 55
# speedup vs baseline: 1.0014x; 1.0014x over previous
"""Trainium2 Bass kernel for DiagonalSelectiveCell.

Problem:
    delta = sigmoid(x @ Wd^T + b_delta)        [T,B,D]
    cand  = x @ Wx^T + b                       [T,B,D]
    gate  = silu(x @ Wg^T + b_gate)            [T,B,D]
    scan over t:  h_t = (1-delta_t) * h_{t-1} + delta_t * tanh(cand_t + r_h*h_{t-1})
    output = h_seq * gate;  h = concat([h0], h_seq)
Returns (output [T,B,D], h [T+1,B,D]).

Strategy (8 NeuronCores, batch-parallel):
  - Shard B=16 across 8 cores (2 rows each), replicate weights. No collectives.
  - Host pre-transposes x to [B_local, D, T] so the GEMMs need no on-device
    transpose: channels live on SBUF partitions, time on the free axis.
  - For the staged inputs r_h == 0, so the recurrence is a first-order LINEAR
    scan per (b, d) lane:  h_t = a_t*h_{t-1} + u_t  with a = 1-delta,
    u = delta*tanh(cand).  That maps 1:1 onto the hardware
    `tensor_tensor_scan` instruction (one instruction scans 2048 steps for
    128 lanes).  A nonzero r_h falls back to an exact numpy path.
  - Per core: for each batch row and each 128-channel tile, fp32r matmuls
    accumulate z in PSUM (K=1024 in 8 chunks, N=512 token tiles), ACT applies
    sigmoid/tanh (one table set, no table swaps; silu is computed as
    z*sigmoid(z) with a DVE scalar_tensor_tensor), DVE runs the scan and the
    output gating, and results DMA out in [D, T] layout (host transposes back).
"""

import numpy as np

T, B, D = 2048, 16, 1024
NCORES = 8
BL = B // NCORES  # batch rows per core
P = 128           # partition tile (channels)
NT = 512          # token tile (PSUM bank / moving free dim)

_NC_CACHE = {}


def _build_nc(t, d, bl, nt):
    import concourse.mybir as mybir
    from concourse import bacc
    from concourse.tile import TileContext

    F32 = mybir.dt.float32
    F32R = mybir.dt.float32r
    AF = mybir.ActivationFunctionType
    OP = mybir.AluOpType

    kd_n = d // P   # contraction chunks
    ke_n = d // P   # output-channel tiles
    ntok = t // nt  # token tiles

    nc = bacc.Bacc()
    xt = nc.declare_dram_parameter("xt", [bl, d, t], F32R, isOutput=False)
    wT = nc.declare_dram_parameter("wT", [3, d, d], F32R, isOutput=False)
    bias = nc.declare_dram_parameter("bias", [4, d], F32, isOutput=False)
    h0t = nc.declare_dram_parameter("h0t", [bl, d], F32, isOutput=False)
    outT = nc.declare_dram_parameter("outT", [bl, d, t], F32, isOutput=True)
    hT = nc.declare_dram_parameter("hT", [bl, d, t + 1], F32, isOutput=True)

    with TileContext(nc) as tc:
        with (
            tc.tile_pool(name="xpool", bufs=1) as xpool,
            tc.tile_pool(name="wpool", bufs=2) as wpool,
            tc.tile_pool(name="spool", bufs=2) as spool,
            tc.tile_pool(name="epool", bufs=2) as epool,
            tc.tile_pool(name="pspool", bufs=2, space="PSUM") as pspool,
        ):
            # bias/h0 tables loaded once: [P, ke] with element (p, ke) =
            # vec[ke*P + p]; per-e-tile slices are [P, 1] scalar APs
            btab = []
            for j in range(4):
                bt = spool.tile([P, ke_n], F32, tag=f"btab{j}", name=f"btab{j}", bufs=1)
                nc.sync.dma_start(out=bt[:], in_=bias[j].rearrange("(ke p) -> p ke", p=P))
                btab.append(bt)
            h0tab = []
            for bb in range(bl):
                ht0 = spool.tile([P, ke_n], F32, tag=f"h0tab{bb}", name=f"h0tab{bb}", bufs=1)
                nc.sync.dma_start(out=ht0[:], in_=h0t[bb].rearrange("(ke p) -> p ke", p=P))
                h0tab.append(ht0)
            for b in range(bl):
                x_sb = [
                    xpool.tile([P, t], F32R, tag=f"x{kd}", name=f"x{kd}", bufs=(2 if kd < 5 else 1))
                    for kd in range(kd_n)
                ]
                for ke in range(ke_n):
                    es = slice(ke * P, (ke + 1) * P)
                    w_sb = []
                    wvs = []
                    for w in range(3):
                        wtile = wpool.tile([P, kd_n * P], F32R, tag=f"w{w}")
                        wvs.append(wtile[:].rearrange("p (kd e) -> p kd e", e=P))
                        w_sb.append(wtile)
                    # Issue order matters: the sync HWDGE ring drains FIFO, so
                    # interleave per-kd weight chunks (needed first by the
                    # matmuls) with the big x chunks instead of queuing 8.4MB
                    # of x ahead of them.
                    if ke > 1:
                        # steady state: one ring slot per weight matrix (the
                        # per-dma fixed cost dominates over transfer size)
                        for w in range(3):
                            nc.sync.dma_start(
                                out=wvs[w][:],
                                in_=wT[w, :, es].rearrange("(kd p) e -> p kd e", p=P),
                            )
                    for kd in range(kd_n):
                        if ke <= 1:
                            for w in range(3):
                                # during the x burst the sync ring is taken;
                                # stream fine-grained weight chunks via SWDGE
                                nc.gpsimd.dma_start(out=wvs[w][:, kd, :], in_=wT[w, kd * P:(kd + 1) * P, es])
                    if ke == 0:
                        # x pieces kd-major: arrival order matches the
                        # kd-outer matmul sweep; per-token-tile pieces so
                        # each matmul unblocks as soon as its piece lands
                        for kd in range(kd_n):
                            for ntk in range(ntok):
                                nc.sync.dma_start(
                                    out=x_sb[kd][:, ntk * nt:(ntk + 1) * nt],
                                    in_=xt[b, kd * P:(kd + 1) * P, ntk * nt:(ntk + 1) * nt],
                                )
                    bd = btab[0][:, ke:ke + 1]
                    nbd = btab[1][:, ke:ke + 1]
                    bc = btab[2][:, ke:ke + 1]
                    bg = btab[3][:, ke:ke + 1]
                    h0_sb = h0tab[b][:, ke:ke + 1]

                    d_sb = epool.tile([P, t], F32, tag="d")
                    a_sb = epool.tile([P, t], F32, tag="a")
                    u_sb = epool.tile([P, t], F32, tag="u")
                    h_sb = epool.tile([P, t + 1], F32, tag="h")

                    def epilogue(w, ntk, ps, fuse_out=True):
                        ts_ = slice(ntk * nt, (ntk + 1) * nt)
                        if w == 0:
                            nc.scalar.activation(d_sb[:, ts_], ps[:], AF.Sigmoid, bias=bd, scale=1.0)
                            nc.scalar.activation(a_sb[:, ts_], ps[:], AF.Sigmoid, bias=nbd, scale=-1.0)
                        elif w == 1:
                            nc.scalar.activation(u_sb[:, ts_], ps[:], AF.Tanh, bias=bc, scale=1.0)
                            nc.vector.tensor_mul(u_sb[:, ts_], u_sb[:, ts_], d_sb[:, ts_])
                        else:
                            g_nt = epool.tile([P, nt], F32, tag="g", name="g_nt")
                            o_nt = epool.tile([P, nt], F32, tag="o", name="o_nt")
                            nc.scalar.activation(g_nt[:], ps[:], AF.Sigmoid, bias=bg, scale=1.0)
                            # gate = (z + b_gate) * sigmoid(z + b_gate)  (= silu)
                            nc.vector.scalar_tensor_tensor(
                                g_nt[:], ps[:], bg, g_nt[:],
                                op0=OP.add, op1=OP.mult,
                            )
                            # output slice pipelines right behind the gate
                            hs_ = slice(ntk * nt + 1, (ntk + 1) * nt + 1)
                            nc.vector.tensor_mul(o_nt[:], h_sb[:, hs_], g_nt[:])
                            nc.gpsimd.dma_start(out=outT[b, es, ts_], in_=o_nt[:])

                    def mm(ps, w, kd, ntk):
                        nc.tensor.matmul(
                            ps[:],
                            lhsT=w_sb[w][:, kd * P:(kd + 1) * P],
                            rhs=x_sb[kd][:, ntk * nt:(ntk + 1) * nt],
                            start=(kd == 0),
                            stop=(kd == kd_n - 1),
                        )

                    # w order: 0=delta(Wd), 1=cand(Wx), 2=gate(Wg)
                    w_list = (0, 1, 2)
                    for w in w_list:
                        # kd-outer: the same 128x128 weight tile feeds all 4
                        # token tiles (amortizes LDWEIGHTS), and the first
                        # matmul only needs the first x/w chunks in SBUF.
                        pss = [
                            pspool.tile([P, nt], F32, tag=f"ps{i}", name=f"ps{i}")
                            for i in range(ntok)
                        ]
                        if w == 2 and b == bl - 1 and ke == ke_n - 1:
                            # final pass of the kernel: ntk-outer staggers the
                            # four groups' completion so only the last token
                            # tile's epilogue chain remains in the drain tail
                            for ntk in range(ntok):
                                for kd in range(kd_n):
                                    mm(pss[ntk], w, kd, ntk)
                                epilogue(w, ntk, pss[ntk])
                        else:
                            for kd in range(kd_n):
                                for ntk in range(ntok):
                                    mm(pss[ntk], w, kd, ntk)
                            for ntk in range(ntok):
                                epilogue(w, ntk, pss[ntk])
                        if w == 1:
                            # scan as soon as a,u are complete (overlaps the
                            # gate matmul pass on PE); h_sb[:, 0] carries h0
                            nc.vector.tensor_copy(h_sb[:, 0:1], h0_sb)
                            nc.vector.tensor_tensor_scan(
                                h_sb[:, 1:], a_sb[:], u_sb[:], h0_sb,
                                op0=OP.mult, op1=OP.add,
                            )
                    nc.gpsimd.dma_start(out=hT[b, es, :], in_=h_sb[:])
    nc.compile()
    return nc


def _get_nc(t=T, d=D, bl=BL, nt=NT):
    key = (t, d, bl, nt)
    if key not in _NC_CACHE:
        _NC_CACHE[key] = _build_nc(t, d, bl, nt)
    return _NC_CACHE[key]


def _marshal_inputs(x, h0, W_x, W_delta, W_gate, b, b_delta, b_gate):
    """Build the per-core input dicts (host-side shard + transpose)."""
    wT = np.ascontiguousarray(
        np.stack([np.asarray(W_delta).T, np.asarray(W_x).T, np.asarray(W_gate).T])
    ).astype(np.float32, copy=False)
    bias = np.ascontiguousarray(
        np.stack([b_delta, -np.asarray(b_delta), b, b_gate])
    ).astype(np.float32, copy=False)
    in_maps = []
    for c in range(NCORES):
        xs = np.ascontiguousarray(
            np.asarray(x)[:, c * BL:(c + 1) * BL, :].transpose(1, 2, 0)
        ).astype(np.float32, copy=False)
        h0s = np.ascontiguousarray(np.asarray(h0)[c * BL:(c + 1) * BL, :]).astype(
            np.float32, copy=False
        )
        in_maps.append({"xt": xs, "wT": wT, "bias": bias, "h0t": h0s})
    return in_maps


def _gather_outputs(results):
    output = np.empty((T, B, D), np.float32)
    h = np.empty((T + 1, B, D), np.float32)
    for c in range(NCORES):
        output[:, c * BL:(c + 1) * BL, :] = results[c]["outT"].transpose(2, 0, 1)
        h[:, c * BL:(c + 1) * BL, :] = results[c]["hT"].transpose(2, 0, 1)
    return output, h


def _run_device(in_maps, trace=False):
    from concourse.bass_utils import run_bass_kernel_spmd

    nc = _get_nc()
    return run_bass_kernel_spmd(
        nc, in_maps, core_ids=list(range(NCORES)), trace=trace
    )


def _numpy_fallback(x, h0, W_x, r_h, W_delta, W_gate, b, b_delta, b_gate):
    x = np.asarray(x, np.float32)
    delta = 1.0 / (1.0 + np.exp(-(np.einsum("tbd,ed->tbe", x, W_delta) + b_delta)))
    cand_x = np.einsum("tbd,ed->tbe", x, W_x) + b
    zg = np.einsum("tbd,ed->tbe", x, W_gate) + b_gate
    gate = zg / (1.0 + np.exp(-zg))
    h_seq = np.empty_like(delta)
    h_prev = np.asarray(h0, np.float32)
    for ti in range(x.shape[0]):
        cand = np.tanh(cand_x[ti] + np.asarray(r_h) * h_prev)
        h_prev = (1.0 - delta[ti]) * h_prev + delta[ti] * cand
        h_seq[ti] = h_prev
    output = h_seq * gate
    h = np.concatenate([np.asarray(h0, np.float32)[None], h_seq], axis=0)
    return output.astype(np.float32), h.astype(np.float32)


def kernel(x, h0, W_x, r_h, W_delta, W_gate, b, b_delta, b_gate):
    x = np.asarray(x, np.float32)
    assert x.shape == (T, B, D), f"unexpected x shape {x.shape}"
    if np.any(np.asarray(r_h) != 0):
        # recurrence is only linear (device-scannable) when r_h == 0
        return _numpy_fallback(x, h0, W_x, r_h, W_delta, W_gate, b, b_delta, b_gate)
    in_maps = _marshal_inputs(x, h0, W_x, W_delta, W_gate, b, b_delta, b_gate)
    res = _run_device(in_maps, trace=False)
    return _gather_outputs(res.results)


# revision 60
# speedup vs baseline: 1.0062x; 1.0048x over previous
"""Trainium2 Bass kernel for DiagonalSelectiveCell.

Problem:
    delta = sigmoid(x @ Wd^T + b_delta)        [T,B,D]
    cand  = x @ Wx^T + b                       [T,B,D]
    gate  = silu(x @ Wg^T + b_gate)            [T,B,D]
    scan over t:  h_t = (1-delta_t) * h_{t-1} + delta_t * tanh(cand_t + r_h*h_{t-1})
    output = h_seq * gate;  h = concat([h0], h_seq)
Returns (output [T,B,D], h [T+1,B,D]).

Strategy (8 NeuronCores, batch-parallel):
  - Shard B=16 across 8 cores (2 rows each), replicate weights. No collectives.
  - Host pre-transposes x to [B_local, D, T] so the GEMMs need no on-device
    transpose: channels live on SBUF partitions, time on the free axis.
  - For the staged inputs r_h == 0, so the recurrence is a first-order LINEAR
    scan per (b, d) lane:  h_t = a_t*h_{t-1} + u_t  with a = 1-delta,
    u = delta*tanh(cand).  That maps 1:1 onto the hardware
    `tensor_tensor_scan` instruction (one instruction scans 2048 steps for
    128 lanes).  A nonzero r_h falls back to an exact numpy path.
  - Per core: for each batch row and each 128-channel tile, fp32r matmuls
    accumulate z in PSUM (K=1024 in 8 chunks, N=512 token tiles), ACT applies
    sigmoid/tanh (one table set, no table swaps; silu is computed as
    z*sigmoid(z) with a DVE scalar_tensor_tensor), DVE runs the scan and the
    output gating, and results DMA out in [D, T] layout (host transposes back).
"""

import numpy as np

T, B, D = 2048, 16, 1024
NCORES = 8
BL = B // NCORES  # batch rows per core
P = 128           # partition tile (channels)
NT = 512          # token tile (PSUM bank / moving free dim)

_NC_CACHE = {}


def _build_nc(t, d, bl, nt):
    import concourse.mybir as mybir
    from concourse import bacc
    from concourse.tile import TileContext

    F32 = mybir.dt.float32
    F32R = mybir.dt.float32r
    AF = mybir.ActivationFunctionType
    OP = mybir.AluOpType

    kd_n = d // P   # contraction chunks
    ke_n = d // P   # output-channel tiles
    ntok = t // nt  # token tiles

    nc = bacc.Bacc()
    xt = nc.declare_dram_parameter("xt", [bl, d, t], F32R, isOutput=False)
    wT = nc.declare_dram_parameter("wT", [3, d, d], F32R, isOutput=False)
    bias = nc.declare_dram_parameter("bias", [4, d], F32, isOutput=False)
    h0t = nc.declare_dram_parameter("h0t", [bl, d], F32, isOutput=False)
    outT = nc.declare_dram_parameter("outT", [bl, d, t], F32, isOutput=True)
    hT = nc.declare_dram_parameter("hT", [bl, d, t + 1], F32, isOutput=True)

    with TileContext(nc) as tc:
        with (
            tc.tile_pool(name="xpool", bufs=1) as xpool,
            tc.tile_pool(name="wpool", bufs=2) as wpool,
            tc.tile_pool(name="spool", bufs=2) as spool,
            tc.tile_pool(name="epool", bufs=2) as epool,
            tc.tile_pool(name="pspool", bufs=2, space="PSUM") as pspool,
        ):
            # bias/h0 tables loaded once: [P, ke] with element (p, ke) =
            # vec[ke*P + p]; per-e-tile slices are [P, 1] scalar APs
            btab = []
            for j in range(4):
                bt = spool.tile([P, ke_n], F32, tag=f"btab{j}", name=f"btab{j}", bufs=1)
                nc.sync.dma_start(out=bt[:], in_=bias[j].rearrange("(ke p) -> p ke", p=P))
                btab.append(bt)
            h0tab = []
            for bb in range(bl):
                ht0 = spool.tile([P, ke_n], F32, tag=f"h0tab{bb}", name=f"h0tab{bb}", bufs=1)
                nc.sync.dma_start(out=ht0[:], in_=h0t[bb].rearrange("(ke p) -> p ke", p=P))
                h0tab.append(ht0)
            for b in range(bl):
                x_sb = [
                    xpool.tile([P, t], F32R, tag=f"x{kd}", name=f"x{kd}", bufs=(2 if kd < 5 else 1))
                    for kd in range(kd_n)
                ]
                for ke in range(ke_n):
                    es = slice(ke * P, (ke + 1) * P)
                    w_sb = []
                    wvs = []
                    for w in range(3):
                        wtile = wpool.tile([P, kd_n * P], F32R, tag=f"w{w}")
                        wvs.append(wtile[:].rearrange("p (kd e) -> p kd e", e=P))
                        w_sb.append(wtile)
                    # Issue order matters: the sync HWDGE ring drains FIFO, so
                    # interleave per-kd weight chunks (needed first by the
                    # matmuls) with the big x chunks instead of queuing 8.4MB
                    # of x ahead of them.
                    if ke > 1:
                        # steady state: one ring slot per weight matrix (the
                        # per-dma fixed cost dominates over transfer size)
                        for w in range(3):
                            nc.sync.dma_start(
                                out=wvs[w][:],
                                in_=wT[w, :, es].rearrange("(kd p) e -> p kd e", p=P),
                            )
                    for kd in range(kd_n):
                        if ke <= 1:
                            for w in range(3):
                                # during the x burst the sync ring is taken;
                                # stream fine-grained weight chunks via SWDGE
                                nc.gpsimd.dma_start(out=wvs[w][:, kd, :], in_=wT[w, kd * P:(kd + 1) * P, es])
                    if ke == 0:
                        # x pieces kd-major: arrival order matches the
                        # kd-outer matmul sweep; per-token-tile pieces so
                        # each matmul unblocks as soon as its piece lands
                        for kd in range(kd_n):
                            for ntk in range(ntok):
                                nc.sync.dma_start(
                                    out=x_sb[kd][:, ntk * nt:(ntk + 1) * nt],
                                    in_=xt[b, kd * P:(kd + 1) * P, ntk * nt:(ntk + 1) * nt],
                                )
                    bd = btab[0][:, ke:ke + 1]
                    nbd = btab[1][:, ke:ke + 1]
                    bc = btab[2][:, ke:ke + 1]
                    bg = btab[3][:, ke:ke + 1]
                    h0_sb = h0tab[b][:, ke:ke + 1]

                    d_sb = epool.tile([P, t], F32, tag="d")
                    a_sb = epool.tile([P, t], F32, tag="a")
                    u_sb = epool.tile([P, t], F32, tag="u")
                    h_sb = epool.tile([P, t + 1], F32, tag="h")

                    def epilogue(w, ntk, ps, fuse_out=True):
                        ts_ = slice(ntk * nt, (ntk + 1) * nt)
                        if w == 0:
                            nc.scalar.activation(d_sb[:, ts_], ps[:], AF.Sigmoid, bias=bd, scale=1.0)
                            nc.scalar.activation(a_sb[:, ts_], ps[:], AF.Sigmoid, bias=nbd, scale=-1.0)
                        elif w == 1:
                            nc.scalar.activation(u_sb[:, ts_], ps[:], AF.Tanh, bias=bc, scale=1.0)
                            nc.vector.tensor_mul(u_sb[:, ts_], u_sb[:, ts_], d_sb[:, ts_])
                        else:
                            g_nt = epool.tile([P, nt], F32, tag="g", name="g_nt")
                            o_nt = epool.tile([P, nt], F32, tag="o", name="o_nt")
                            nc.scalar.activation(g_nt[:], ps[:], AF.Sigmoid, bias=bg, scale=1.0)
                            # gate = (z + b_gate) * sigmoid(z + b_gate)  (= silu)
                            nc.vector.scalar_tensor_tensor(
                                g_nt[:], ps[:], bg, g_nt[:],
                                op0=OP.add, op1=OP.mult,
                            )
                            # output slice pipelines right behind the gate
                            hs_ = slice(ntk * nt + 1, (ntk + 1) * nt + 1)
                            nc.vector.tensor_mul(o_nt[:], h_sb[:, hs_], g_nt[:])
                            nc.gpsimd.dma_start(out=outT[b, es, ts_], in_=o_nt[:])

                    def mm(ps, w, kd, ntk):
                        nc.tensor.matmul(
                            ps[:],
                            lhsT=w_sb[w][:, kd * P:(kd + 1) * P],
                            rhs=x_sb[kd][:, ntk * nt:(ntk + 1) * nt],
                            start=(kd == 0),
                            stop=(kd == kd_n - 1),
                        )

                    # w order: 0=delta(Wd), 1=cand(Wx), 2=gate(Wg)
                    w_list = (0, 1, 2)
                    for w in w_list:
                        # kd-outer: the same 128x128 weight tile feeds all 4
                        # token tiles (amortizes LDWEIGHTS), and the first
                        # matmul only needs the first x/w chunks in SBUF.
                        pss = [
                            pspool.tile([P, nt], F32, tag=f"ps{i}", name=f"ps{i}")
                            for i in range(ntok)
                        ]
                        if b == bl - 1 and ke == ke_n - 1:
                            # final pass of the kernel: ntk-outer staggers the
                            # four groups' completion so only the last token
                            # tile's epilogue chain remains in the drain tail
                            for ntk in range(ntok):
                                for kd in range(kd_n):
                                    mm(pss[ntk], w, kd, ntk)
                                epilogue(w, ntk, pss[ntk])
                        else:
                            for kd in range(kd_n):
                                for ntk in range(ntok):
                                    mm(pss[ntk], w, kd, ntk)
                            for ntk in range(ntok):
                                epilogue(w, ntk, pss[ntk])
                        if w == 1:
                            # scan as soon as a,u are complete (overlaps the
                            # gate matmul pass on PE); h_sb[:, 0] carries h0
                            nc.vector.tensor_copy(h_sb[:, 0:1], h0_sb)
                            nc.vector.tensor_tensor_scan(
                                h_sb[:, 1:], a_sb[:], u_sb[:], h0_sb,
                                op0=OP.mult, op1=OP.add,
                            )
                    nc.gpsimd.dma_start(out=hT[b, es, :], in_=h_sb[:])
    nc.compile()
    return nc


def _get_nc(t=T, d=D, bl=BL, nt=NT):
    key = (t, d, bl, nt)
    if key not in _NC_CACHE:
        _NC_CACHE[key] = _build_nc(t, d, bl, nt)
    return _NC_CACHE[key]


def _marshal_inputs(x, h0, W_x, W_delta, W_gate, b, b_delta, b_gate):
    """Build the per-core input dicts (host-side shard + transpose)."""
    wT = np.ascontiguousarray(
        np.stack([np.asarray(W_delta).T, np.asarray(W_x).T, np.asarray(W_gate).T])
    ).astype(np.float32, copy=False)
    bias = np.ascontiguousarray(
        np.stack([b_delta, -np.asarray(b_delta), b, b_gate])
    ).astype(np.float32, copy=False)
    in_maps = []
    for c in range(NCORES):
        xs = np.ascontiguousarray(
            np.asarray(x)[:, c * BL:(c + 1) * BL, :].transpose(1, 2, 0)
        ).astype(np.float32, copy=False)
        h0s = np.ascontiguousarray(np.asarray(h0)[c * BL:(c + 1) * BL, :]).astype(
            np.float32, copy=False
        )
        in_maps.append({"xt": xs, "wT": wT, "bias": bias, "h0t": h0s})
    return in_maps


def _gather_outputs(results):
    output = np.empty((T, B, D), np.float32)
    h = np.empty((T + 1, B, D), np.float32)
    for c in range(NCORES):
        output[:, c * BL:(c + 1) * BL, :] = results[c]["outT"].transpose(2, 0, 1)
        h[:, c * BL:(c + 1) * BL, :] = results[c]["hT"].transpose(2, 0, 1)
    return output, h


def _run_device(in_maps, trace=False):
    from concourse.bass_utils import run_bass_kernel_spmd

    nc = _get_nc()
    return run_bass_kernel_spmd(
        nc, in_maps, core_ids=list(range(NCORES)), trace=trace
    )


def _numpy_fallback(x, h0, W_x, r_h, W_delta, W_gate, b, b_delta, b_gate):
    x = np.asarray(x, np.float32)
    delta = 1.0 / (1.0 + np.exp(-(np.einsum("tbd,ed->tbe", x, W_delta) + b_delta)))
    cand_x = np.einsum("tbd,ed->tbe", x, W_x) + b
    zg = np.einsum("tbd,ed->tbe", x, W_gate) + b_gate
    gate = zg / (1.0 + np.exp(-zg))
    h_seq = np.empty_like(delta)
    h_prev = np.asarray(h0, np.float32)
    for ti in range(x.shape[0]):
        cand = np.tanh(cand_x[ti] + np.asarray(r_h) * h_prev)
        h_prev = (1.0 - delta[ti]) * h_prev + delta[ti] * cand
        h_seq[ti] = h_prev
    output = h_seq * gate
    h = np.concatenate([np.asarray(h0, np.float32)[None], h_seq], axis=0)
    return output.astype(np.float32), h.astype(np.float32)


def kernel(x, h0, W_x, r_h, W_delta, W_gate, b, b_delta, b_gate):
    x = np.asarray(x, np.float32)
    assert x.shape == (T, B, D), f"unexpected x shape {x.shape}"
    if np.any(np.asarray(r_h) != 0):
        # recurrence is only linear (device-scannable) when r_h == 0
        return _numpy_fallback(x, h0, W_x, r_h, W_delta, W_gate, b, b_delta, b_gate)
    in_maps = _marshal_inputs(x, h0, W_x, W_delta, W_gate, b, b_delta, b_gate)
    res = _run_device(in_maps, trace=False)
    return _gather_outputs(res.results)


# revision 63
# speedup vs baseline: 1.0105x; 1.0042x over previous
"""Trainium2 Bass kernel for DiagonalSelectiveCell.

Problem:
    delta = sigmoid(x @ Wd^T + b_delta)        [T,B,D]
    cand  = x @ Wx^T + b                       [T,B,D]
    gate  = silu(x @ Wg^T + b_gate)            [T,B,D]
    scan over t:  h_t = (1-delta_t) * h_{t-1} + delta_t * tanh(cand_t + r_h*h_{t-1})
    output = h_seq * gate;  h = concat([h0], h_seq)
Returns (output [T,B,D], h [T+1,B,D]).

Strategy (8 NeuronCores, batch-parallel):
  - Shard B=16 across 8 cores (2 rows each), replicate weights. No collectives.
  - Host pre-transposes x to [B_local, D, T] so the GEMMs need no on-device
    transpose: channels live on SBUF partitions, time on the free axis.
  - For the staged inputs r_h == 0, so the recurrence is a first-order LINEAR
    scan per (b, d) lane:  h_t = a_t*h_{t-1} + u_t  with a = 1-delta,
    u = delta*tanh(cand).  That maps 1:1 onto the hardware
    `tensor_tensor_scan` instruction (one instruction scans 2048 steps for
    128 lanes).  A nonzero r_h falls back to an exact numpy path.
  - Per core: for each batch row and each 128-channel tile, fp32r matmuls
    accumulate z in PSUM (K=1024 in 8 chunks, N=512 token tiles), ACT applies
    sigmoid/tanh (one table set, no table swaps; silu is computed as
    z*sigmoid(z) with a DVE scalar_tensor_tensor), DVE runs the scan and the
    output gating, and results DMA out in [D, T] layout (host transposes back).
"""

import numpy as np

T, B, D = 2048, 16, 1024
NCORES = 8
BL = B // NCORES  # batch rows per core
P = 128           # partition tile (channels)
NT = 512          # token tile (PSUM bank / moving free dim)

_NC_CACHE = {}


def _build_nc(t, d, bl, nt):
    import concourse.mybir as mybir
    from concourse import bacc
    from concourse.tile import TileContext

    F32 = mybir.dt.float32
    F32R = mybir.dt.float32r
    AF = mybir.ActivationFunctionType
    OP = mybir.AluOpType

    kd_n = d // P   # contraction chunks
    ke_n = d // P   # output-channel tiles
    ntok = t // nt  # token tiles

    nc = bacc.Bacc()
    xt = nc.declare_dram_parameter("xt", [bl, d, t], F32R, isOutput=False)
    wT = nc.declare_dram_parameter("wT", [3, d, d], F32R, isOutput=False)
    bias = nc.declare_dram_parameter("bias", [4, d], F32, isOutput=False)
    h0t = nc.declare_dram_parameter("h0t", [bl, d], F32, isOutput=False)
    outT = nc.declare_dram_parameter("outT", [bl, d, t], F32, isOutput=True)
    hT = nc.declare_dram_parameter("hT", [bl, d, t + 1], F32, isOutput=True)

    with TileContext(nc) as tc:
        with (
            tc.tile_pool(name="xpool", bufs=1) as xpool,
            tc.tile_pool(name="wpool", bufs=2) as wpool,
            tc.tile_pool(name="spool", bufs=2) as spool,
            tc.tile_pool(name="epool", bufs=2) as epool,
            tc.tile_pool(name="pspool", bufs=2, space="PSUM") as pspool,
        ):
            # bias/h0 tables loaded once: [P, ke] with element (p, ke) =
            # vec[ke*P + p]; per-e-tile slices are [P, 1] scalar APs
            btab = []
            for j in range(4):
                bt = spool.tile([P, ke_n], F32, tag=f"btab{j}", name=f"btab{j}", bufs=1)
                nc.sync.dma_start(out=bt[:], in_=bias[j].rearrange("(ke p) -> p ke", p=P))
                btab.append(bt)
            h0tab = []
            for bb in range(bl):
                ht0 = spool.tile([P, ke_n], F32, tag=f"h0tab{bb}", name=f"h0tab{bb}", bufs=1)
                nc.sync.dma_start(out=ht0[:], in_=h0t[bb].rearrange("(ke p) -> p ke", p=P))
                h0tab.append(ht0)
            for b in range(bl):
                x_sb = [
                    xpool.tile([P, t], F32R, tag=f"x{kd}", name=f"x{kd}", bufs=(2 if kd < 5 else 1))
                    for kd in range(kd_n)
                ]
                for ke in range(ke_n):
                    es = slice(ke * P, (ke + 1) * P)
                    w_sb = []
                    wvs = []
                    for w in range(3):
                        wtile = wpool.tile([P, kd_n * P], F32R, tag=f"w{w}")
                        wvs.append(wtile[:].rearrange("p (kd e) -> p kd e", e=P))
                        w_sb.append(wtile)
                    # Issue order matters: the sync HWDGE ring drains FIFO, so
                    # interleave per-kd weight chunks (needed first by the
                    # matmuls) with the big x chunks instead of queuing 8.4MB
                    # of x ahead of them.
                    if ke > 1:
                        # steady state: one ring slot per weight matrix (the
                        # per-dma fixed cost dominates over transfer size)
                        for w in range(3):
                            nc.sync.dma_start(
                                out=wvs[w][:],
                                in_=wT[w, :, es].rearrange("(kd p) e -> p kd e", p=P),
                            )
                    for kd in range(kd_n):
                        if ke <= 1:
                            for w in range(3):
                                # during the x burst the sync ring is taken;
                                # stream fine-grained weight chunks via SWDGE
                                nc.gpsimd.dma_start(out=wvs[w][:, kd, :], in_=wT[w, kd * P:(kd + 1) * P, es])
                    if ke == 0:
                        # x pieces kd-major: arrival order matches the
                        # kd-outer matmul sweep; per-token-tile pieces so
                        # each matmul unblocks as soon as its piece lands
                        for kd in range(kd_n):
                            for ntk in range(ntok):
                                nc.sync.dma_start(
                                    out=x_sb[kd][:, ntk * nt:(ntk + 1) * nt],
                                    in_=xt[b, kd * P:(kd + 1) * P, ntk * nt:(ntk + 1) * nt],
                                )
                    bd = btab[0][:, ke:ke + 1]
                    nbd = btab[1][:, ke:ke + 1]
                    bc = btab[2][:, ke:ke + 1]
                    bg = btab[3][:, ke:ke + 1]
                    h0_sb = h0tab[b][:, ke:ke + 1]

                    d_sb = epool.tile([P, t], F32, tag="d")
                    a_sb = epool.tile([P, t], F32, tag="a")
                    u_sb = epool.tile([P, t], F32, tag="u")
                    h_sb = epool.tile([P, t + 1], F32, tag="h")

                    def epilogue(w, ntk, ps, fuse_out=True, oeng=None):
                        ts_ = slice(ntk * nt, (ntk + 1) * nt)
                        if w == 0:
                            nc.scalar.activation(d_sb[:, ts_], ps[:], AF.Sigmoid, bias=bd, scale=1.0)
                            nc.scalar.activation(a_sb[:, ts_], ps[:], AF.Sigmoid, bias=nbd, scale=-1.0)
                        elif w == 1:
                            nc.scalar.activation(u_sb[:, ts_], ps[:], AF.Tanh, bias=bc, scale=1.0)
                            nc.vector.tensor_mul(u_sb[:, ts_], u_sb[:, ts_], d_sb[:, ts_])
                        else:
                            g_nt = epool.tile([P, nt], F32, tag="g", name="g_nt")
                            o_nt = epool.tile([P, nt], F32, tag="o", name="o_nt")
                            nc.scalar.activation(g_nt[:], ps[:], AF.Sigmoid, bias=bg, scale=1.0)
                            # gate = (z + b_gate) * sigmoid(z + b_gate)  (= silu)
                            nc.vector.scalar_tensor_tensor(
                                g_nt[:], ps[:], bg, g_nt[:],
                                op0=OP.add, op1=OP.mult,
                            )
                            # output slice pipelines right behind the gate
                            hs_ = slice(ntk * nt + 1, (ntk + 1) * nt + 1)
                            nc.vector.tensor_mul(o_nt[:], h_sb[:, hs_], g_nt[:])
                            (oeng or nc.gpsimd).dma_start(out=outT[b, es, ts_], in_=o_nt[:])

                    def mm(ps, w, kd, ntk):
                        nc.tensor.matmul(
                            ps[:],
                            lhsT=w_sb[w][:, kd * P:(kd + 1) * P],
                            rhs=x_sb[kd][:, ntk * nt:(ntk + 1) * nt],
                            start=(kd == 0),
                            stop=(kd == kd_n - 1),
                        )

                    # w order: 0=delta(Wd), 1=cand(Wx), 2=gate(Wg)
                    w_list = (0, 1, 2)
                    for w in w_list:
                        # kd-outer: the same 128x128 weight tile feeds all 4
                        # token tiles (amortizes LDWEIGHTS), and the first
                        # matmul only needs the first x/w chunks in SBUF.
                        pss = [
                            pspool.tile([P, nt], F32, tag=f"ps{i}", name=f"ps{i}")
                            for i in range(ntok)
                        ]
                        if b == bl - 1 and ke == ke_n - 1:
                            # final pass of the kernel: ntk-outer staggers the
                            # four groups' completion so only the last token
                            # tile's epilogue chain remains in the drain tail
                            for ntk in range(ntok):
                                for kd in range(kd_n):
                                    mm(pss[ntk], w, kd, ntk)
                                # kernel tail: the sync ring is idle here
                                epilogue(w, ntk, pss[ntk], oeng=nc.sync)
                        else:
                            for kd in range(kd_n):
                                for ntk in range(ntok):
                                    mm(pss[ntk], w, kd, ntk)
                            for ntk in range(ntok):
                                epilogue(w, ntk, pss[ntk])
                        if w == 1:
                            # scan as soon as a,u are complete (overlaps the
                            # gate matmul pass on PE); h_sb[:, 0] carries h0
                            nc.vector.tensor_copy(h_sb[:, 0:1], h0_sb)
                            nc.vector.tensor_tensor_scan(
                                h_sb[:, 1:], a_sb[:], u_sb[:], h0_sb,
                                op0=OP.mult, op1=OP.add,
                            )
                    heng = nc.sync if (b == bl - 1 and ke == ke_n - 1) else nc.gpsimd
                    heng.dma_start(out=hT[b, es, :], in_=h_sb[:])
    nc.compile()
    return nc


def _get_nc(t=T, d=D, bl=BL, nt=NT):
    key = (t, d, bl, nt)
    if key not in _NC_CACHE:
        _NC_CACHE[key] = _build_nc(t, d, bl, nt)
    return _NC_CACHE[key]


def _marshal_inputs(x, h0, W_x, W_delta, W_gate, b, b_delta, b_gate):
    """Build the per-core input dicts (host-side shard + transpose)."""
    wT = np.ascontiguousarray(
        np.stack([np.asarray(W_delta).T, np.asarray(W_x).T, np.asarray(W_gate).T])
    ).astype(np.float32, copy=False)
    bias = np.ascontiguousarray(
        np.stack([b_delta, -np.asarray(b_delta), b, b_gate])
    ).astype(np.float32, copy=False)
    in_maps = []
    for c in range(NCORES):
        xs = np.ascontiguousarray(
            np.asarray(x)[:, c * BL:(c + 1) * BL, :].transpose(1, 2, 0)
        ).astype(np.float32, copy=False)
        h0s = np.ascontiguousarray(np.asarray(h0)[c * BL:(c + 1) * BL, :]).astype(
            np.float32, copy=False
        )
        in_maps.append({"xt": xs, "wT": wT, "bias": bias, "h0t": h0s})
    return in_maps


def _gather_outputs(results):
    output = np.empty((T, B, D), np.float32)
    h = np.empty((T + 1, B, D), np.float32)
    for c in range(NCORES):
        output[:, c * BL:(c + 1) * BL, :] = results[c]["outT"].transpose(2, 0, 1)
        h[:, c * BL:(c + 1) * BL, :] = results[c]["hT"].transpose(2, 0, 1)
    return output, h


def _run_device(in_maps, trace=False):
    from concourse.bass_utils import run_bass_kernel_spmd

    nc = _get_nc()
    return run_bass_kernel_spmd(
        nc, in_maps, core_ids=list(range(NCORES)), trace=trace
    )


def _numpy_fallback(x, h0, W_x, r_h, W_delta, W_gate, b, b_delta, b_gate):
    x = np.asarray(x, np.float32)
    delta = 1.0 / (1.0 + np.exp(-(np.einsum("tbd,ed->tbe", x, W_delta) + b_delta)))
    cand_x = np.einsum("tbd,ed->tbe", x, W_x) + b
    zg = np.einsum("tbd,ed->tbe", x, W_gate) + b_gate
    gate = zg / (1.0 + np.exp(-zg))
    h_seq = np.empty_like(delta)
    h_prev = np.asarray(h0, np.float32)
    for ti in range(x.shape[0]):
        cand = np.tanh(cand_x[ti] + np.asarray(r_h) * h_prev)
        h_prev = (1.0 - delta[ti]) * h_prev + delta[ti] * cand
        h_seq[ti] = h_prev
    output = h_seq * gate
    h = np.concatenate([np.asarray(h0, np.float32)[None], h_seq], axis=0)
    return output.astype(np.float32), h.astype(np.float32)


def kernel(x, h0, W_x, r_h, W_delta, W_gate, b, b_delta, b_gate):
    x = np.asarray(x, np.float32)
    assert x.shape == (T, B, D), f"unexpected x shape {x.shape}"
    if np.any(np.asarray(r_h) != 0):
        # recurrence is only linear (device-scannable) when r_h == 0
        return _numpy_fallback(x, h0, W_x, r_h, W_delta, W_gate, b, b_delta, b_gate)
    in_maps = _marshal_inputs(x, h0, W_x, W_delta, W_gate, b, b_delta, b_gate)
    res = _run_device(in_maps, trace=False)
    return _gather_outputs(res.results)
